# revision 1
# baseline (speedup 1.0000x reference)
"""Causal self-attention (B=2, T=4096, C=768, H=12, D=64) on 8 trn2 cores.

Sharding: batch*heads across cores. Core c handles batch c//4 and heads
3*(c%4) .. 3*(c%4)+2. Each core computes the QKV projection for its head
slice, full causal attention for those heads, and a partial output
projection (its heads' rows of w_out). The host sums the 4 partials per
batch and adds b_out.

On-core layouts (matmul operands float32r - fp32 data consumed at full
PE rate with ~1e-4 rounding; the PE rounds on read, so fp32 bits are
DMA'd straight into f32r tiles):
  xT      [C, T]   input, pre-transposed on host
  qT/kT   [64, T]  packed so q_h and k_h share a partition base
                   (matmul requires lhsT/rhs base alignment)
  v_aug   [T, 256] natural-layout v with a ones column per head at
                   col h*65+64 (so P@V also yields softmax denominators)
  scoresT [k, q]   psum; exp on ACT; causal mask via gpsimd affine_select
  outT    [65, q]  psum accumulation over k tiles; row 64 = sum(exp)

Packed [128, T] sbuf tiles (rows 0:64 | 64:128):
  tA = [qT_h0 | qT_h1]   tB = [kT_h0 | kT_h1]
  tC = [outT_h0 | outT_h1] tD = [outT_h2 | qT_h2] tE = [- | kT_h2]
(outT_h0/h1 share a tile so the output projection contracts 128 rows
per matmul; h1's normalize result is DMA-bounced to partition base 64)

The projection is emitted chunk-by-chunk inside the attention q-block
loop (chunk qb produces exactly the 512 columns attention block qb
needs), so the scalar engine's exp stream starts as soon as the first
chunk lands instead of after the whole projection.
"""

import numpy as np

import concourse.bass as bass
import concourse.mybir as mybir
import concourse.tile as tile
from concourse import bacc
from concourse.bass_utils import run_bass_kernel_spmd

B, T, C = 2, 4096, 768
NH, D = 12, 64
HPC = 3  # heads per core
NCORES = 8
P = 128
QB = 512           # q block == projection chunk
NQB = T // QB      # 8
NKT = T // P       # 32 k tiles
F32 = mybir.dt.float32
F32R = mybir.dt.float32r

_CACHE = {}


def _build_nc():
    nc = bacc.Bacc(
        "TRN2",
        target_bir_lowering=False,
        debug=False,
        enable_asserts=False,
        num_devices=NCORES,
    )
    # wqk columns: [q_h0 q_h1 | k_h0 k_h1 | q_h2 | k_h2]
    xT = nc.dram_tensor("xT", [C, T], F32R, kind="ExternalInput")
    wqk = nc.dram_tensor("wqk", [C, 2 * HPC * D], F32R, kind="ExternalInput")
    wv = nc.dram_tensor("wv", [C, 256], F32R, kind="ExternalInput")
    wo = nc.dram_tensor("wo", [HPC * D, C], F32R, kind="ExternalInput")
    out = nc.dram_tensor("out", [T, C], F32, kind="ExternalOutput")

    with tile.TileContext(nc) as tc:
        _emit(tc, nc, xT.ap(), wqk.ap(), wv.ap(), wo.ap(), out.ap())
    nc.compile()
    return nc


def _emit(tc, nc, xT, wqk, wv, wo, out):
    import contextlib

    ctx = contextlib.ExitStack()
    with ctx:
        # ---- persistent sbuf ----
        persist = ctx.enter_context(tc.tile_pool(name="persist", bufs=1))
        packs = [
            persist.tile([P, T], F32R, tag=f"pk{m}", name=f"pk{m}") for m in range(5)
        ]
        tA, tB, tC, tD, tE = packs
        vaug = persist.tile([P, NKT, 256], F32R, tag="vaug")
        wqk_sb = persist.tile([P, 6, 2 * HPC * D], F32R, tag="wqk")
        wv_sb = persist.tile([P, 6, 256], F32R, tag="wv")
        wo01_sb = persist.tile([P, C], F32R, tag="wo01")
        wo2_sb = persist.tile([D, C], F32R, tag="wo2")
        ones_f32 = persist.tile([P, D], F32, tag="onesf32")

        nc.sync.dma_start(out=wqk_sb[:], in_=wqk.rearrange("(co p) n -> p co n", p=P))
        nc.sync.dma_start(out=wv_sb[:], in_=wv.rearrange("(co p) n -> p co n", p=P))
        nc.sync.dma_start(out=wo01_sb[:], in_=wo[0:P, :])
        nc.sync.dma_start(out=wo2_sb[:], in_=wo[P : P + D, :])
        nc.gpsimd.memset(ones_f32[:], 1.0)

        def qT(h):
            return (tA[0:D], tA[D:P], tD[D:P])[h]

        def kT(h):
            return (tB[0:D], tB[D:P], tE[D:P])[h]

        # ---- fused projection + attention loop ----
        # psum budget (8 banks): p1 2 + scores 4 + outT 2
        with (
            tc.tile_pool(name="xchunks", bufs=2) as xpool,
            tc.tile_pool(name="p1psum", bufs=2, space="PSUM") as p1psum,
            tc.tile_pool(name="spsum", bufs=3, space="PSUM") as spool,
            tc.tile_pool(name="p3psum", bufs=1, space="PSUM") as p3psum,
            tc.tile_pool(name="opsum", bufs=2, space="PSUM") as opool,
            tc.tile_pool(name="exps", bufs=3) as epool,
            tc.tile_pool(name="smalls", bufs=4) as rpool,
            tc.tile_pool(name="dscratch", bufs=4, space="DRAM") as dpool,
        ):
            for qb in range(NQB):
                qsl = slice(qb * QB, (qb + 1) * QB)

                # -- projection chunk qb: columns [qb*512, qb*512+512) --
                xt = xpool.tile([P, 6, QB], F32R, tag="xt")
                nc.sync.dma_start(
                    out=xt[:], in_=xT[:, qsl].rearrange("(co p) t -> p co t", p=P)
                )
                for ci in range(3):
                    ps = p1psum.tile([P, QB], F32, tag="p1", name=f"p1_{qb}_{ci}")
                    for c6 in range(6):
                        nc.tensor.matmul(
                            ps[:],
                            wqk_sb[:, c6, ci * P : (ci + 1) * P],
                            xt[:, c6, :],
                            start=(c6 == 0),
                            stop=(c6 == 5),
                        )
                    if ci < 2:
                        dst = (tA, tB)[ci]
                        nc.vector.tensor_copy(out=dst[:, qsl], in_=ps[:])
                    else:
                        # chain 2 = [qT_h2 | kT_h2] at psum base 0; the packed
                        # destinations live at partition base 64, which only a
                        # DMA can reach (engines cannot cross partitions)
                        stg = xpool.tile([P, QB], F32R, tag="stg")
                        nc.vector.tensor_copy(out=stg[:], in_=ps[:])
                        nc.sync.dma_start(out=tD[D:P, qsl], in_=stg[0:D, :])
                        nc.sync.dma_start(out=tE[D:P, qsl], in_=stg[D:P, :])
                for half in range(QB // P):
                    ktv = qb * (QB // P) + half
                    ps2 = p1psum.tile([P, QB], F32, tag="p1", name=f"p1v_{qb}_{half}")
                    for c6 in range(6):
                        nc.tensor.matmul(
                            ps2[:, 0:256],
                            xt[:, c6, half * P : (half + 1) * P],
                            wv_sb[:, c6, :],
                            start=(c6 == 0),
                            stop=(c6 == 5),
                        )
                    nc.vector.tensor_copy(out=vaug[:, ktv, :], in_=ps2[:, 0:256])
                # restore the ones columns the v copies just overwrote
                for h in range(HPC):
                    nc.vector.tensor_copy(
                        out=vaug[:, qb * (QB // P) : (qb + 1) * (QB // P),
                                 h * (D + 1) + D],
                        in_=ones_f32[:, 0 : QB // P],
                    )

                # -- attention for q block qb --
                for h in range(HPC):
                    nkt = 4 * qb + 4
                    outp = opool.tile([D + 1, QB], F32, tag="outT")
                    for kt in range(nkt):
                        co = max(0, P * (kt - 4 * qb))
                        sp = spool.tile([P, QB], F32, tag="scores")
                        nc.tensor.matmul(
                            sp[:, co:],
                            kT(h)[:, kt * P : (kt + 1) * P],
                            qT(h)[:, qb * QB + co : (qb + 1) * QB],
                            start=True,
                            stop=True,
                        )
                        ex = epool.tile([P, QB], F32R, tag="ex")
                        nc.scalar.activation(
                            out=ex[:, co:],
                            in_=sp[:, co:],
                            func=mybir.ActivationFunctionType.Exp,
                            scale=float(D) ** -0.5,
                        )
                        if kt >= 4 * qb:  # diagonal band: causal mask
                            nc.gpsimd.affine_select(
                                out=ex[:, co:],
                                in_=ex[:, co:],
                                compare_op=mybir.AluOpType.is_ge,
                                fill=0.0,
                                base=0,
                                pattern=[[1, QB - co]],
                                channel_multiplier=-1,
                            )
                        nc.tensor.matmul(
                            outp[:, co:],
                            vaug[:, kt, h * (D + 1) : (h + 1) * (D + 1)],
                            ex[:, co:],
                            start=(kt == 0),
                            stop=(kt == nkt - 1),
                        )
                    # softmax denominators: reciprocal of outp row 64 stays at
                    # partition base 64 (engines cannot cross partitions); a
                    # partition-broadcast DMA then fans it out across 0:64
                    recip = rpool.tile([D + 1, QB], F32, tag="recip")
                    nc.vector.reciprocal(
                        out=recip[D : D + 1, :], in_=outp[D : D + 1, :]
                    )
                    # partition-broadcast via DRAM bounce (SBUF sources must
                    # have nonzero partition step; DRAM reads may broadcast)
                    dsc = dpool.tile([1, QB], F32, tag="dsc")
                    nc.sync.dma_start(out=dsc[:], in_=recip[D : D + 1, :])
                    bcs = rpool.tile([D, QB], F32, tag="bcs")
                    nc.gpsimd.dma_start(
                        out=bcs[:],
                        in_=bass.AP(
                            tensor=dsc.tensor,
                            offset=dsc.offset,
                            ap=[[0, D]] + list(dsc.ap[-1:]),
                        ),
                    )
                    if h == 0:
                        nc.vector.tensor_mul(
                            out=tC[0:D, qsl], in0=outp[0:D, :], in1=bcs[:]
                        )
                    elif h == 2:
                        nc.vector.tensor_mul(
                            out=tD[0:D, qsl], in0=outp[0:D, :], in1=bcs[:]
                        )
                    else:
                        # h1 lives at partition base 64 of tC; engines cannot
                        # cross partitions, so normalize into a staging tile
                        # and DMA-bounce it up
                        ot = rpool.tile([D, QB], F32R, tag="otmp", bufs=2)
                        nc.vector.tensor_mul(
                            out=ot[:], in0=outp[0:D, :], in1=bcs[:]
                        )
                        nc.sync.dma_start(out=tC[D:P, qsl], in_=ot[:])

                # -- output projection for this q block (tail of the loop;
                # psum comes from the p1 tag so the bank budget stays at 8) --
                for tt in range(qb * (QB // P), (qb + 1) * (QB // P)):
                    tsl = slice(tt * P, (tt + 1) * P)
                    so = rpool.tile([P, C], F32, tag="p3out", bufs=2)
                    for noff, nsz in ((0, 512), (512, 256)):
                        po = p3psum.tile(
                            [P, QB], F32, tag="p3", name=f"po_{tt}_{noff}"
                        )
                        nc.tensor.matmul(
                            po[:, :nsz],
                            tC[:, tsl],
                            wo01_sb[:, noff : noff + nsz],
                            start=True,
                            stop=False,
                        )
                        nc.tensor.matmul(
                            po[:, :nsz],
                            tD[0:D, tsl],
                            wo2_sb[:, noff : noff + nsz],
                            start=False,
                            stop=True,
                        )
                        nc.vector.tensor_copy(
                            out=so[:, noff : noff + nsz], in_=po[:, :nsz]
                        )
                    nc.sync.dma_start(out=out[tsl, :], in_=so[:])


def _get_nc():
    if "nc" not in _CACHE:
        _CACHE["nc"] = _build_nc()
    return _CACHE["nc"]


def _shard_inputs(x, w_qkv, w_out):
    """Build per-core input maps."""
    x = np.asarray(x, dtype=np.float32)
    w_qkv = np.asarray(w_qkv, dtype=np.float32)
    w_out = np.asarray(w_out, dtype=np.float32)
    xTs = [np.ascontiguousarray(x[b].T) for b in range(B)]
    in_maps = []
    for c in range(NCORES):
        b = c // 4
        heads = [HPC * (c % 4) + i for i in range(HPC)]
        q = [w_qkv[:, h * D : (h + 1) * D] for h in heads]
        k = [w_qkv[:, C + h * D : C + (h + 1) * D] for h in heads]
        wqk = np.concatenate([q[0], q[1], k[0], k[1], q[2], k[2]], axis=1)
        wv = np.zeros((C, 256), dtype=np.float32)
        for i, h in enumerate(heads):
            wv[:, i * (D + 1) : i * (D + 1) + D] = w_qkv[
                :, 2 * C + h * D : 2 * C + (h + 1) * D
            ]
        wo = np.concatenate(
            [w_out[h * D : (h + 1) * D, :] for h in heads], axis=0
        )
        in_maps.append(
            {
                "xT": xTs[b],
                "wqk": np.ascontiguousarray(wqk),
                "wv": wv,
                "wo": np.ascontiguousarray(wo),
            }
        )
    return in_maps


def kernel(x, w_qkv, w_out, b_out):
    nc = _get_nc()
    in_maps = _shard_inputs(x, w_qkv, w_out)
    res = run_bass_kernel_spmd(nc, in_maps, core_ids=list(range(NCORES)))
    b_out = np.asarray(b_out, dtype=np.float32)
    outs = []
    for b in range(B):
        acc = res.results[4 * b]["out"].astype(np.float32).copy()
        for c in range(4 * b + 1, 4 * b + 4):
            acc += res.results[c]["out"]
        outs.append(acc + b_out[None, :])
    return np.stack(outs, axis=0)



# revision 3
# speedup vs baseline: 2.0204x; 2.0204x over previous
"""Causal self-attention (B=2, T=4096, C=768, H=12, D=64) on 8 trn2 cores.

Sharding: batch*heads across cores. Core c handles batch c//4 and heads
3*(c%4)..3*(c%4)+2. Each core computes the QKV projection for its head
slice, full causal attention for those heads, and a partial output
projection; the host sums the 4 partials per batch and adds b_out.

Design notes (single-shot NTFF-profiled evolution 717us -> 354us):
  - all matmul operands are fp16 (fp32r streams 4B/elem through the PE's
    SBUF port at ~1 col per 2 warm cycles; fp16 runs at full PE rate and
    its weight loads engage FWL). PSUM accumulation stays fp32; max |s|
    ~6 so exp<=e^6 and fp16's 65504 range is safe; rel_err ~4e-4.
  - exp is batched over pairs of k-tiles ([128,1024] PSUM spans) to
    amortize the ~293ns ACT instruction overhead.
  - softmax denominator: the PV matmul's ones-column yields sum(exp) on
    PSUM partition 64; it is broadcast to partitions 0:64 by a rank-1
    matmul against a ones column (engines cannot cross partitions; v1's
    DRAM-bounce DMA round trip cost ~5us latency per block+head), then
    reciprocal_approx_fast + multiply normalize.
  - software-pipelined emission: the attention stream is intrinsically
    ACT-paced (per k-tile pair the PE does 4x512-col matmuls ~0.85us vs
    ACT's ~1.15us exp), and PE micro-idles make the PE's HAM clock gate
    oscillate between 2.4 and 1.2 GHz (half rate ~50% of the time in the
    v2 trace). The PE instruction stream is static FIFO, so filler must
    be placed in program order: projection chains for chunk qb+1 are
    emitted interleaved between attention pairs of block qb, and ALL
    output projections are deferred to the last two blocks where the
    exp-paced slack is largest (early blocks are PE-bound). This keeps
    the PE ~100% busy and the clock gate warm end to end.

On-core layout (fp16 unless noted):
  xT      [C, T]       input, pre-transposed on host
  tA/tB   [128, T]     [qT_h0|qT_h1] / [kT_h0|kT_h1] (h1 at partition 64)
  tQ2/tK2 [64, T]      head 2 q/k, both at base 0 (two half chains)
  vaug    [128,32,256] natural-layout v + per-head ones column at
                       h*65+64 (PV matmul then also yields denominators)
  oT_h    [64, T]      normalized attention output, transposed
  scoresT [k, q]       PSUM pairs [128,1024]; exp on ACT; causal mask via
                       gpsimd affine_select on the diagonal band
"""

import numpy as np

import concourse.bass as bass
import concourse.mybir as mybir
import concourse.tile as tile
from concourse import bacc
from concourse.bass_utils import run_bass_kernel_spmd

B, T, C = 2, 4096, 768
NH, D = 12, 64
HPC = 3
NCORES = 8
P = 128
QB = 512
NQB = T // QB      # 8
NKT = T // P       # 32
F32 = mybir.dt.float32
F32R = mybir.dt.float32r
F16 = mybir.dt.float16

_CACHE = {}


def _declare_io(nc):
    # wqk columns: [q0|q1 (128) | k0|k1 (128) | q2 (64) | k2 (64)]
    xT = nc.dram_tensor("xT", [C, T], F16, kind="ExternalInput")
    wqk = nc.dram_tensor("wqk", [C, 2 * HPC * D], F16, kind="ExternalInput")
    wv = nc.dram_tensor("wv", [C, 256], F16, kind="ExternalInput")
    wo = nc.dram_tensor("wo", [HPC * D, C], F16, kind="ExternalInput")
    out = nc.dram_tensor("out", [T, C], F32, kind="ExternalOutput")
    return xT.ap(), wqk.ap(), wv.ap(), wo.ap(), out.ap()


def _build_nc():
    nc = bacc.Bacc(
        "TRN2",
        target_bir_lowering=False,
        debug=False,
        enable_asserts=False,
        num_devices=NCORES,
    )
    aps = _declare_io(nc)
    with tile.TileContext(nc) as tc:
        _emit(tc, nc, *aps)
    nc.compile()
    return nc


def _emit(tc, nc, xT, wqk, wv, wo, out):
    import contextlib

    ctx = contextlib.ExitStack()
    with ctx:
        persist = ctx.enter_context(tc.tile_pool(name="persist", bufs=1))
        tA = persist.tile([P, T], F16, tag="tA")
        tB = persist.tile([P, T], F16, tag="tB")
        tQ2 = persist.tile([D, T], F16, tag="tQ2")
        tK2 = persist.tile([D, T], F16, tag="tK2")
        # h0 rows 0:64, h1 rows 64:128 (bounced in via DMA) -> the output
        # projection contracts 128 rows in one matmul for those two heads
        oTP = persist.tile([P, T], F16, tag="oTP")
        oT2 = persist.tile([D, T], F16, tag="oT2")
        vaug = persist.tile([P, NKT, 256], F16, tag="vaug")
        wqk_sb = persist.tile([P, 6, 2 * HPC * D], F16, tag="wqk")
        wv_sb = persist.tile([P, 6, 256], F16, tag="wv")
        wo01_sb = persist.tile([P, C], F16, tag="wo01")
        wo2_sb = persist.tile([D, C], F16, tag="wo2")
        ones_f16 = persist.tile([P, D], F16, tag="ones16")

        # weights on the SP HWDGE ring; x chunks ride the ACT HWDGE ring so
        # the first projection chain isn't serialized behind the weights
        nc.sync.dma_start(out=wqk_sb[:], in_=wqk.rearrange("(co p) n -> p co n", p=P))
        nc.sync.dma_start(out=wv_sb[:], in_=wv.rearrange("(co p) n -> p co n", p=P))
        nc.sync.dma_start(out=wo01_sb[:], in_=wo[0:P, :])
        nc.sync.dma_start(out=wo2_sb[:], in_=wo[P : P + D, :])
        nc.gpsimd.memset(ones_f16[:], 1.0)

        def qT(h):
            return (tA[0:D], tA[D:P], tQ2[:])[h]

        def kT(h):
            return (tB[0:D], tB[D:P], tK2[:])[h]

        # psum budget (8 banks): p1 2 + scores 4 + outT(+bc) 2
        with (
            tc.tile_pool(name="xchunks", bufs=2) as xpool,
            tc.tile_pool(name="p1psum", bufs=2, space="PSUM") as p1psum,
            tc.tile_pool(name="spsum", bufs=2, space="PSUM") as spool,
            tc.tile_pool(name="opsum", bufs=2, space="PSUM") as opool,
            tc.tile_pool(name="exps", bufs=3) as epool,
            tc.tile_pool(name="smalls", bufs=2) as rpool,
            tc.tile_pool(name="outs", bufs=2) as sopool,
        ):

            def proj_steps(qb):
                """QKV projection for chunk qb; yields between matmul groups."""
                qsl = slice(qb * QB, (qb + 1) * QB)
                xt = xpool.tile([P, 6, QB], F16, tag="xt", name=f"xt{qb}")
                nc.scalar.dma_start(
                    out=xt[:], in_=xT[:, qsl].rearrange("(co p) t -> p co t", p=P)
                )
                yield
                # c2 computes [q2|k2]; k2 (partitions 64:128) is bounced to
                # tK2's partitions 0:64 by an SBUF->SBUF DMA (engines can't
                # cross partitions; a second half-chain would cost 6 more
                # 512-col matmuls)
                for ci, csl in enumerate(
                    (slice(0, P), slice(P, 2 * P), slice(2 * P, 3 * P))
                ):
                    ps = p1psum.tile([P, QB], F32, tag="p1", name=f"p1_{qb}_{ci}")
                    for c6 in range(6):
                        nc.tensor.matmul(
                            ps[:],
                            wqk_sb[:, c6, csl],
                            xt[:, c6, :],
                            start=(c6 == 0),
                            stop=(c6 == 5),
                        )
                        yield
                    if ci < 2:
                        nc.vector.tensor_copy(
                            out=(tA, tB)[ci][:, qsl], in_=ps[:]
                        )
                    else:
                        nc.vector.tensor_copy(out=tQ2[:, qsl], in_=ps[0:D, :])
                        stg = rpool.tile([P, QB], F16, tag="stg")
                        nc.vector.tensor_copy(out=stg[D:P, :], in_=ps[D:P, :])
                        nc.sync.dma_start(out=tK2[:, qsl], in_=stg[D:P, :])
                    yield
                for half in range(QB // P):
                    ktv = qb * (QB // P) + half
                    ps2 = p1psum.tile([P, QB], F32, tag="p1", name=f"p1v_{qb}_{half}")
                    for c6 in range(6):
                        nc.tensor.matmul(
                            ps2[:, 0:256],
                            xt[:, c6, half * P : (half + 1) * P],
                            wv_sb[:, c6, :],
                            start=(c6 == 0),
                            stop=(c6 == 5),
                        )
                        yield
                    nc.vector.tensor_copy(out=vaug[:, ktv, :], in_=ps2[:, 0:256])
                    yield
                for h in range(HPC):
                    nc.vector.tensor_copy(
                        out=vaug[:, qb * (QB // P) : (qb + 1) * (QB // P),
                                 h * (D + 1) + D],
                        in_=ones_f16[:, 0 : QB // P],
                    )
                yield

            def outproj_steps(qb):
                """Output projection for block qb; yields between chunks."""
                for half in range(2):
                    so = sopool.tile(
                        [P, 2, C], F32, tag="so", name=f"so{qb}_{half}"
                    )
                    for t2 in range(2):
                        tt4 = 2 * half + t2
                        tsl = slice(qb * QB + tt4 * P, qb * QB + (tt4 + 1) * P)
                        for noff, nsz in ((0, 512), (512, 256)):
                            po = p1psum.tile(
                                [P, QB], F32, tag="p1",
                                name=f"po_{qb}_{tt4}_{noff}",
                            )
                            nc.tensor.matmul(
                                po[:, :nsz],
                                oTP[:, tsl],
                                wo01_sb[:, noff : noff + nsz],
                                start=True,
                                stop=False,
                            )
                            yield
                            nc.tensor.matmul(
                                po[:, :nsz],
                                oT2[:, tsl],
                                wo2_sb[:, noff : noff + nsz],
                                start=False,
                                stop=True,
                            )
                            yield
                            nc.vector.tensor_copy(
                                out=so[:, t2, noff : noff + nsz], in_=po[:, :nsz]
                            )
                            yield
                    hsl = slice(qb * QB + half * 2 * P, qb * QB + (half + 1) * 2 * P)
                    nc.sync.dma_start(
                        out=out[hsl, :].rearrange("(n p) c -> p n c", p=P),
                        in_=so[:],
                    )
                    yield

            def attn_steps(qb):
                """Attention for block qb; yields after each pair / normalize."""
                qsl = slice(qb * QB, (qb + 1) * QB)
                nkt = 4 * qb + 4
                for h in range(HPC):
                    outp = opool.tile(
                        [D + 1, QB], F32, tag="outT", name=f"outp_{qb}_{h}"
                    )
                    for pr in range(nkt // 2):
                        kt0 = 2 * pr
                        co = max(0, P * (kt0 - 4 * qb))
                        sp = spool.tile(
                            [P, 2, QB], F32, tag="sc", name=f"sc_{qb}_{h}_{pr}"
                        )
                        ex = epool.tile([P, 2, QB], F16, tag="ex")
                        for j in range(2):
                            kt = kt0 + j
                            nc.tensor.matmul(
                                sp[:, j, co:],
                                kT(h)[:, kt * P : (kt + 1) * P],
                                qT(h)[:, qb * QB + co : (qb + 1) * QB],
                                start=True,
                                stop=True,
                            )
                        nc.scalar.activation(
                            out=ex[:, :, co:],
                            in_=sp[:, :, co:],
                            func=mybir.ActivationFunctionType.Exp,
                            scale=float(D) ** -0.5,
                        )
                        if kt0 >= 4 * qb:  # diagonal band: causal mask
                            for j in range(2):
                                kt = kt0 + j
                                nc.gpsimd.affine_select(
                                    out=ex[:, j, co:],
                                    in_=ex[:, j, co:],
                                    compare_op=mybir.AluOpType.is_ge,
                                    fill=0.0,
                                    base=-P * (kt - 4 * qb) + co,
                                    pattern=[[1, QB - co]],
                                    channel_multiplier=-1,
                                )
                        yield
                        for j in range(2):
                            kt = kt0 + j
                            cj = max(0, P * (kt - 4 * qb))
                            nc.tensor.matmul(
                                outp[:, cj:],
                                vaug[:, kt, h * (D + 1) : (h + 1) * (D + 1)],
                                ex[:, j, cj:],
                                start=(kt == 0),
                                stop=(kt == nkt - 1),
                            )
                        yield
                    # evacuate PV+denoms to SBUF (frees the PSUM slot fast),
                    # then broadcast the denom row to partitions 0:64 via a
                    # rank-1 matmul against ones, reciprocal, normalize.
                    ou = rpool.tile([D + 1, QB], F16, tag="ou")
                    nc.vector.tensor_copy(out=ou[:], in_=outp[:])
                    yield
                    bc = opool.tile([D, QB], F32, tag="outT", name=f"bc_{qb}_{h}")
                    nc.tensor.matmul(
                        bc[:],
                        ones_f16[D : D + 1, 0:D],
                        ou[D : D + 1, :],
                        start=True,
                        stop=True,
                    )
                    bcs = rpool.tile([D, QB], F32, tag="bcs")
                    nc.vector.reciprocal_approx_fast(out=bcs[:], in_=bc[:])
                    if h == 0:
                        nc.vector.tensor_mul(
                            out=oTP[0:D, qsl], in0=ou[0:D, :], in1=bcs[:]
                        )
                    elif h == 2:
                        nc.vector.tensor_mul(
                            out=oT2[:, qsl], in0=ou[0:D, :], in1=bcs[:]
                        )
                    else:
                        # h1 lives at oTP partitions 64:128, which only a
                        # DMA can reach from partitions 0:64
                        o1s = rpool.tile([D, QB], F16, tag="o1s")
                        nc.vector.tensor_mul(
                            out=o1s[:], in0=ou[0:D, :], in1=bcs[:]
                        )
                        nc.sync.dma_start(out=oTP[D:P, qsl], in_=o1s[:])
                    yield

            def drain(g):
                for _ in g:
                    pass

            # ---- pipelined emission ----
            # attention(qb) is the pacer. Early blocks are PE-bound (the
            # projection for chunk qb+1 alone exceeds their ACT slack), so
            # ALL output projections are deferred to the last two blocks,
            # whose exp-paced slack would otherwise leave the PE idling
            # (and its HAM clock gate dropping to half rate).
            import itertools

            drain(proj_steps(0))
            for qb in range(NQB):
                parts = []
                n_fill = 0
                if qb + 1 < NQB:
                    parts.append(proj_steps(qb + 1))
                    n_fill += 51
                if qb == NQB - 2:
                    parts += [outproj_steps(j) for j in range(0, 3)]
                    n_fill += 3 * 26
                elif qb == NQB - 1:
                    parts += [outproj_steps(j) for j in range(3, NQB - 1)]
                    n_fill += (NQB - 4) * 26
                filler = itertools.chain(*parts)
                n_attn = sum(1 for _ in _attn_yield_count(qb))
                credit = -2.0  # let the xt DMA land before pulling proj MMs
                step = n_fill / max(1, n_attn)
                for _ in attn_steps(qb):
                    credit += step
                    while credit >= 1.0:
                        if next(filler, None) is None:
                            credit = -1e9
                            break
                        credit -= 1.0
                drain(filler)
            drain(outproj_steps(NQB - 1))


def _attn_yield_count(qb):
    nkt = 4 * qb + 4
    for h in range(HPC):
        for pr in range(nkt // 2):
            yield
            yield
        yield
        yield


def _get_nc():
    if "nc" not in _CACHE:
        _CACHE["nc"] = _build_nc()
    return _CACHE["nc"]


def _shard_inputs(x, w_qkv, w_out):
    """Build per-core input maps (fp16 operands)."""
    x = np.asarray(x, dtype=np.float32)
    w_qkv = np.asarray(w_qkv, dtype=np.float32)
    w_out = np.asarray(w_out, dtype=np.float32)
    xTs = [np.ascontiguousarray(x[b].T.astype(np.float16)) for b in range(B)]
    in_maps = []
    for c in range(NCORES):
        b = c // 4
        heads = [HPC * (c % 4) + i for i in range(HPC)]
        q = [w_qkv[:, h * D : (h + 1) * D] for h in heads]
        k = [w_qkv[:, C + h * D : C + (h + 1) * D] for h in heads]
        wqk = np.concatenate([q[0], q[1], k[0], k[1], q[2], k[2]], axis=1)
        wv = np.zeros((C, 256), dtype=np.float32)
        for i, h in enumerate(heads):
            wv[:, i * (D + 1) : i * (D + 1) + D] = w_qkv[
                :, 2 * C + h * D : 2 * C + (h + 1) * D
            ]
        wo = np.concatenate(
            [w_out[h * D : (h + 1) * D, :] for h in heads], axis=0
        )
        in_maps.append(
            {
                "xT": xTs[b],
                "wqk": np.ascontiguousarray(wqk.astype(np.float16)),
                "wv": wv.astype(np.float16),
                "wo": np.ascontiguousarray(wo.astype(np.float16)),
            }
        )
    return in_maps


def kernel(x, w_qkv, w_out, b_out):
    nc = _get_nc()
    in_maps = _shard_inputs(x, w_qkv, w_out)
    res = run_bass_kernel_spmd(nc, in_maps, core_ids=list(range(NCORES)))
    b_out = np.asarray(b_out, dtype=np.float32)
    outs = []
    for b in range(B):
        acc = res.results[4 * b]["out"].astype(np.float32).copy()
        for c in range(4 * b + 1, 4 * b + 4):
            acc += res.results[c]["out"]
        outs.append(acc + b_out[None, :])
    return np.stack(outs, axis=0)
